# revision 40
# baseline (speedup 1.0000x reference)
"""Depthwise 4x4 separable blur (upfirdn2d pad=(2,1)) on 8 TRN2 NeuronCores.

v6 design — fp16 end-to-end on the wire, tensor_tensor W-pass, gpsimd assist:
  - Host pre-transposes each core's image to [HP=128, C, 2, W] fp16 so every
    DMA descriptor is an 8 KB contiguous DRAM run; output uses the same
    layout and the host transposes back to fp32. Halves HBM traffic vs fp32.
  - W-pass on VectorE as two plain tensor_tensor adds (the blur kernel is
    symmetric, so t1 = x[w-2] + x[w+1] and t2 = x[w-1] + x[w] need no
    scalar); fp16 TT runs in the DVE 2x packed mode (~2.3 us per tile op).
    Asymmetric separable kernels fall back to scalar_tensor_tensor (1x).
  - H-pass on TensorE: per channel pair, a 2-bank psum [ep, c, w] accumulates
    8 banded matmuls (2 streams x 2 input h-parities x 2 output parities),
    pair-outer so each pair's ScalarE psum copy frees its banks early.
  - The last pair of every tile group instead pre-combines y = t1 + s*t2 on
    GPSIMD (otherwise idle, own queue) and runs only 4 matmuls — shifts
    ~13 us of TensorE stream time onto GpSimd.
"""

import os
import sys

import numpy as np

for _p in ("/opt/trn_rl_repo", "/root/.axon_site/_ro/trn_rl_repo"):
    if os.path.isdir(_p) and _p not in sys.path:
        sys.path.append(_p)

import concourse.bacc as bacc
import concourse.mybir as mybir
from concourse import tile
from concourse.bass_utils import run_bass_kernel_spmd

B, C, H, W = 8, 128, 256, 256
N_CORES = 8
CG = 8               # channels per inner tile group
HP = H // 2          # 128 h-pairs = partitions
EW = 2 * W           # flat (e, w) extent per (partition, channel) = 512
KS = 4
MM_DT = mybir.dt.float16
IO_DT = mybir.dt.float16


def _build_bands(kern: np.ndarray):
    """Factor flip(kern) = outer(kh, kw); build the 8 parity band matrices."""
    k = np.flip(kern.astype(np.float64), (0, 1))
    u, s, vt = np.linalg.svd(k)
    assert s[1] < 1e-6 * s[0], "blur kernel must be separable"
    kh = u[:, 0] * np.sqrt(s[0])
    kw = vt[0] * np.sqrt(s[0])
    if kh.sum() < 0:
        kh, kw = -kh, -kw
    assert np.allclose(np.outer(kh, kw), k, atol=1e-12 + 1e-7 * np.abs(k).max())
    assert abs(kw[2]) > 1e-12 and abs(kw[3]) > 1e-12
    r1 = float(kw[0] / kw[3])    # t1 = r1 * x[w-2] + x[w+1]  (band scale kw3)
    r2 = float(kw[1] / kw[2])    # t2 = r2 * x[w-1] + x[w]    (band scale kw2)
    s_y = float(kw[2] / kw[3])   # y  = t1 + s_y * t2         (band scale kw3)
    scales = (kw[3], kw[2])      # psum += scale_q * band^T tq

    M = np.zeros((H, H), np.float64)
    for hh in range(H):
        for t in range(KS):
            i = hh + t - 2
            if 0 <= i < H:
                M[i, hh] = kh[t]
    bands = np.zeros((2, 2, 2, HP, HP), np.float64)
    for q in range(2):
        for e in range(2):
            for ep in range(2):
                bands[q, e, ep] = scales[q] * M[e::2, ep::2]
    return bands.reshape(8, HP, HP).astype(np.float16), r1, r2, s_y


def _build_nc(r1: float, r2: float, s_y: float):
    nc = bacc.Bacc("TRN2", target_bir_lowering=False, debug=False,
                   num_devices=N_CORES)
    x = nc.dram_tensor("input", [HP, C, EW], IO_DT,
                       kind="ExternalInput").ap()
    bands = nc.dram_tensor("bands", [8, HP, HP], MM_DT,
                           kind="ExternalInput").ap()
    out = nc.dram_tensor("output", [HP, C, EW], IO_DT,
                         kind="ExternalOutput").ap()
    mult = mybir.AluOpType.mult
    add = mybir.AluOpType.add
    plain = abs(r1 - 1.0) < 1e-9 and abs(r2 - 1.0) < 1e-9

    with tile.TileContext(nc) as tc:
        with (
            tc.tile_pool(name="bands", bufs=1) as bp,
            tc.tile_pool(name="xp", bufs=4) as xpp,
            tc.tile_pool(name="tp", bufs=6) as tpp,
            tc.tile_pool(name="yp", bufs=8) as ypp,
            tc.tile_pool(name="osb", bufs=4) as osb,
            tc.tile_pool(name="ps", bufs=4, space="PSUM") as pp,
        ):
            wm = {}

            def _prep_bands():
                for idx in range(8):
                    br = bp.tile([HP, HP], MM_DT, tag=f"br{idx}")
                    nc.scalar.dma_start(br[:], bands[idx])
                    q, e, ep = idx >> 2, (idx >> 1) & 1, idx & 1
                    wm[q, e, ep] = br

            # Taper first/last groups so pipeline fill and drain are short.
            segs = []
            c = 0
            for cg in [4, 4] + [CG] * ((C - 16) // CG) + [4, 2, 2]:
                segs.append((c, cg))
                c += cg
            assert c == C
            n_seg = len(segs)
            for si, (c0, cg) in enumerate(segs):
                fg = cg * EW
                # The last pair of every tile pre-combines y on GpSimd (its
                # ~3.7us add hides behind the other pairs' matmul streams) —
                # except the drain-tail segments, where GpSimd latency would
                # sit on the critical path.
                npair = cg // 2
                y_pairs = [npair - 1] if si < n_seg - 2 else []
                n_pairs = [pr for pr in range(npair) if pr not in y_pairs]
                xt = xpp.tile([HP, fg], IO_DT, tag="x")
                xf = xt[:]
                nc.sync.dma_start(
                    xf.rearrange("p (c f) -> p c f", c=cg),
                    x[:, c0:c0 + cg, :],
                )
                if not wm:
                    _prep_bands()
                t1 = tpp.tile([HP, fg], MM_DT, tag="t1")
                t2 = tpp.tile([HP, fg], MM_DT, tag="t2")
                t1f, t2f = t1[:], t2[:]
                # Main W-pass (fp16 TT -> DVE 2x packed mode); run-boundary
                # columns come out wrong and are overwritten by the fixups.
                if plain:
                    nc.vector.tensor_tensor(
                        t1f[:, 2:fg - 1], xf[:, 0:fg - 3], xf[:, 3:fg], add)
                    nc.vector.tensor_tensor(
                        t2f[:, 1:fg], xf[:, 0:fg - 1], xf[:, 1:fg], add)
                else:
                    nc.vector.scalar_tensor_tensor(
                        t1f[:, 2:fg - 1], xf[:, 0:fg - 3], r1,
                        xf[:, 3:fg], mult, add)
                    nc.vector.scalar_tensor_tensor(
                        t2f[:, 1:fg], xf[:, 0:fg - 1], r2,
                        xf[:, 1:fg], mult, add)
                # Fixups (strided 4d views over c and both e rows):
                t1e = t1f.rearrange("p (c pr w) -> p c pr w", c=cg, pr=2)
                t2e = t2f.rearrange("p (c pr w) -> p c pr w", c=cg, pr=2)
                xe = xf.rearrange("p (c pr w) -> p c pr w", c=cg, pr=2)
                # t1[w=0,1] = x[w+1] (left pad kills the r1 term)
                nc.vector.tensor_copy(t1e[:, :, :, 0:2], xe[:, :, :, 1:3])
                # t1[w=255] = r1 * x[w-2] (right pad kills the + term)
                nc.vector.tensor_scalar_mul(
                    t1e[:, :, :, W - 1:W], xe[:, :, :, W - 3:W - 2], r1)
                # t2[w=0] = x[w] (left pad kills the r2 term)
                nc.vector.tensor_copy(t2e[:, :, :, 0:1], xe[:, :, :, 0:1])

                t1c = t1f.rearrange("p (c f) -> p c f", c=cg)
                t2c = t2f.rearrange("p (c f) -> p c f", c=cg)
                osbt = osb.tile([HP, fg], IO_DT, tag="o")
                osb4 = osbt[:].rearrange("p (c e w) -> p c e w", c=cg, e=2)
                pss = {}

                def _copy_out(pr):
                    ps4 = pss[pr][:].rearrange(
                        "p (e c w) -> p c e w", e=2, c=2)
                    nc.scalar.copy(osb4[:, pr * 2:pr * 2 + 2, :, :], ps4)

                # The y combine for all y-pairs at once (one DVE 4x scale +
                # one GpSimd add; GP fixed cost amortizes over the batch).
                if y_pairs:
                    ya = y_pairs[0] * 2 * EW
                    yw = len(y_pairs) * 2 * EW
                    ys = ypp.tile([HP, 2 * EW], MM_DT, tag="ys", name="ys")
                    yt = ypp.tile([HP, 2 * EW], MM_DT, tag="y", name="y")
                    nc.vector.tensor_scalar_mul(
                        ys[:, 0:yw], t2f[:, ya:ya + yw], s_y)
                    nc.gpsimd.tensor_tensor(
                        yt[:, 0:yw], ys[:, 0:yw], t1f[:, ya:ya + yw], add)
                    yc = yt[:, 0:yw].rearrange(
                        "p (c f) -> p c f", c=2 * len(y_pairs))
                # Normal pairs, pair-outer: each pair's 8 matmuls run
                # consecutively so its psum copy fires while the next pair
                # streams, freeing the 2-bank psum slot early.
                for pr in n_pairs:
                    pss[pr] = pp.tile([HP, 1024], mybir.dt.float32,
                                      tag="ps", name="ps")
                    lc = pr * 2
                    for ep in (0, 1):
                        for bi, (q, e) in enumerate(
                                ((0, 0), (0, 1), (1, 0), (1, 1))):
                            src = t1c if q == 0 else t2c
                            rhs = src[:, lc:lc + 2, e * W:(e + 1) * W]
                            nc.tensor.matmul(
                                pss[pr][:, ep * 512:(ep + 1) * 512],
                                wm[q, e, ep][:], rhs,
                                start=(bi == 0), stop=(bi == 3))
                    _copy_out(pr)
                # y-pairs: the 4 shared weights each serve the whole block.
                for pr in y_pairs:
                    pss[pr] = pp.tile([HP, 1024], mybir.dt.float32,
                                      tag="ps", name="ps")
                for ep in (0, 1):
                    for bi, e in enumerate((0, 1)):
                        for pr in y_pairs:
                            j = pr - y_pairs[0]
                            rhs = yc[:, 2 * j:2 * j + 2, e * W:(e + 1) * W]
                            nc.tensor.matmul(
                                pss[pr][:, ep * 512:(ep + 1) * 512],
                                wm[0, e, ep][:], rhs,
                                start=(bi == 0), stop=(bi == 1))
                for pr in y_pairs:
                    _copy_out(pr)
                nc.scalar.dma_start(
                    out[:, c0:c0 + cg, :],
                    osbt[:].rearrange("p (c f) -> p c f", c=cg),
                )
    nc.compile()
    return nc


_CACHE = {}


def _get_nc(r1: float, r2: float, s_y: float):
    key = (r1, r2, s_y)
    if key not in _CACHE:
        _CACHE[key] = _build_nc(r1, r2, s_y)
    return _CACHE[key]


def kernel(**inputs) -> np.ndarray:
    x = np.asarray(inputs["input"], dtype=np.float32)
    kern = np.asarray(inputs["kernel"], dtype=np.float32)
    assert x.shape == (B, C, H, W) and kern.shape == (KS, KS)
    bands, r1, r2, s_y = _build_bands(kern)
    nc = _get_nc(r1, r2, s_y)
    in_maps = []
    for i in range(N_CORES):
        xi = (x[i].reshape(C, HP, 2, W).transpose(1, 0, 2, 3)
              .reshape(HP, C, EW).astype(np.float16))
        in_maps.append({"input": xi, "bands": bands})
    res = run_bass_kernel_spmd(nc, in_maps, list(range(N_CORES)))
    global _LAST_RESULTS
    _LAST_RESULTS = res
    outs = []
    for i in range(N_CORES):
        oi = res.results[i]["output"]
        outs.append(oi.reshape(HP, C, 2, W).transpose(1, 0, 2, 3)
                    .reshape(C, H, W).astype(np.float32))
    return np.stack(outs)


if __name__ == "__main__":
    rng = np.random.default_rng(0)
    x = rng.standard_normal((B, C, H, W), dtype=np.float32)
    k1 = np.array([1.0, 3.0, 3.0, 1.0], np.float64)
    k = np.outer(k1, k1)
    k = (k / k.sum() * 4).astype(np.float32)
    y = kernel(input=x, kernel=k)
    print("out", y.shape, y.dtype, float(np.abs(y).max()))
